# revision 10
# baseline (speedup 1.0000x reference)
"""Chamfer loss kernel for Trainium2 (8 NeuronCores, data-parallel over batch).

Problem: pred_seq [8,8192,3] f32, tgt_output [8,8192,3] f32 ->
  chamfer [8] f32, where per batch b:
    d[n,m]   = || pred[b,n] - tgt[b,m] ||_2
    chamfer  = (mean_n min_m d + mean_m min_n d) / 2

Strategy (one batch element per core):
  - d2[n,m] = |p_n|^2 + |t_m|^2 - 2 p.t computed as ONE K=16 fp16 matmul per
    128x512 tile, using an exact fp16 hi/lo split of the coordinates (products
    of fp16 are exact in the PE's fp32 accumulator; measured d2 error ~7e-6).
  - min-before-sqrt: min_m sqrt(x) == sqrt(min_m x), so only the 2x8192 row/col
    minima ever need sqrt.
  - PSUM groups are consumed in pairs: ScalarE stages two 2048-wide f32 PSUM
    groups into one 4096-wide fp16 SBUF tile (pair A lands directly in the row
    running-min buffer), then VectorE runs 2x-rate fp16 tensor-tensor mins for
    the column accumulators and row running min, plus a TT-halving chain for
    the final free-axis row reduction (all at the DVE's 2-elem/cycle limit).
  - Column minima finish with PE transposes + free-axis reductions.
  - Means via ones-matmul partition sum; sqrt on the 2x8192 minima only.

Host side does only format prep: the fp16 hi/lo split + squared norms
(0.5 MFLOP total vs ~3.4 GFLOP on device).
"""

import functools
import sys

if "/opt/trn_rl_repo" not in sys.path:
    sys.path.insert(0, "/opt/trn_rl_repo")

import numpy as np

B = 8
NPTS = 8192
D = 3
K = 16  # augmented contraction dim: 4 slots per coord + 2 norm slots per side
BIG = 60000.0  # > max possible d2 (~200), fits fp16


# ---------------------------------------------------------------------------
# host-side augmentation: exact fp16 hi/lo split
# ---------------------------------------------------------------------------
def _split(x32):
    h = x32.astype(np.float16)
    l = (x32 - h.astype(np.float32)).astype(np.float16)
    return h, l


def _augment(pred, tgt):
    """pred/tgt: [N,3] f32 -> U,V [16,N] fp16 with d2 = (U^T V)[n,m]."""
    n = pred.shape[0]
    U = np.empty((K, n), np.float16)
    V = np.empty((K, n), np.float16)
    for d in range(D):
        hp, lp = _split(pred[:, d])
        ht, lt = _split(tgt[:, d])
        U[4 * d + 0] = hp
        U[4 * d + 1] = hp
        U[4 * d + 2] = lp
        U[4 * d + 3] = lp
        V[4 * d + 0] = -2.0 * ht
        V[4 * d + 1] = -2.0 * lt
        V[4 * d + 2] = -2.0 * ht
        V[4 * d + 3] = -2.0 * lt
    np_p = (pred * pred).sum(axis=1, dtype=np.float32)
    np_t = (tgt * tgt).sum(axis=1, dtype=np.float32)
    h, l = _split(np_p)
    U[12], U[13] = h, l
    V[12], V[13] = 1.0, 1.0
    h, l = _split(np_t)
    U[14], U[15] = 1.0, 1.0
    V[14], V[15] = h, l
    return U, V


# ---------------------------------------------------------------------------
# device program
# ---------------------------------------------------------------------------
def _emit(nc, tc, u_ext, v_ext, out_ext, npts, reps=1):
    from contextlib import nullcontext

    import concourse.tile as tile  # noqa: F401
    from concourse import mybir
    from concourse.masks import make_identity

    FP16 = mybir.dt.float16
    F32 = mybir.dt.float32
    MIN = mybir.AluOpType.min
    ADD = mybir.AluOpType.add
    X = mybir.AxisListType.X

    GROUP = 2048
    NG = npts // GROUP  # col groups
    NRT = npts // 128  # row tiles
    NC4 = GROUP // 512  # matmuls per group

    with (
        tc.tile_pool(name="consts", bufs=1) as consts,
        tc.tile_pool(name="uv", bufs=1) as uv,
        tc.tile_pool(name="acc", bufs=1) as accp,
        tc.tile_pool(name="mins", bufs=1) as minsp,
    ):
        identity = consts.tile([128, 128], FP16)
        make_identity(nc, identity)
        ones = consts.tile([128, 1], F32)
        nc.vector.memset(ones, 1.0)

        u = uv.tile([K, npts], FP16)
        nc.sync.dma_start(out=u, in_=u_ext[:])
        v = uv.tile([K, npts], FP16)
        nc.sync.dma_start(out=v, in_=v_ext[:])

        colacc = accp.tile([128, npts], FP16, tag="colacc", name="colacc")

        rowmins = minsp.tile([128, NRT], F32)
        colmins = minsp.tile([128, NRT], F32)

        rep_cm = tc.For_i(0, reps, 1) if reps > 1 else nullcontext()
        with rep_cm:
            _emit_body(
                nc, tc, v, u, out_ext, colacc, rowmins, colmins, identity, ones, npts
            )


def _emit_body(nc, tc, v, u, out_ext, colacc, rowmins, colmins, identity, ones, npts):
    from concourse import mybir

    FP16 = mybir.dt.float16
    F32 = mybir.dt.float32
    MIN = mybir.AluOpType.min
    ADD = mybir.AluOpType.add
    X = mybir.AxisListType.X

    GROUP = 2048
    NG = npts // GROUP
    NRT = npts // 128
    NC4 = GROUP // 512

    if True:  # preserve indentation structure
        nc.vector.memset(colacc, BIG)

        # ---------------- phase 1: d2 tiles + row/col min accumulation ------
        # ScalarE stages all NG 2048-wide PSUM groups of one row tile into a
        # single npts-wide fp16 SBUF tile. VectorE then needs just ONE
        # full-width 2x-rate tensor-tensor min into the column accumulator,
        # and the row min is a TT-halving chain (2x rate) off the same staged
        # tile + one short 1x reduce. 6 DVE ops per row tile, all at the
        # DVE's 2-elem/cycle crossbar limit.
        with (
            tc.tile_pool(name="psmm", bufs=2, space="PSUM") as psmm,
            tc.tile_pool(name="rows", bufs=3) as rowsp,
            tc.tile_pool(name="red", bufs=3) as redp,
        ):
            for r in range(NRT):
                lhsT = u[:, 128 * r : 128 * (r + 1)]
                rowrun = rowsp.tile([128, npts], FP16, tag="rowrun")
                for g in range(NG):
                    pg = psmm.tile([128, GROUP], F32, tag="mm")
                    for c in range(NC4):
                        nc.tensor.matmul(
                            pg[:, 512 * c : 512 * (c + 1)],
                            lhsT,
                            v[:, GROUP * g + 512 * c : GROUP * g + 512 * (c + 1)],
                            start=True,
                            stop=True,
                        )
                    nc.scalar.copy(rowrun[:, GROUP * g : GROUP * (g + 1)], pg[:])
                # column accumulator (elementwise min across row tiles)
                nc.vector.tensor_tensor(
                    out=colacc[:], in0=rowrun[:], in1=colacc[:], op=MIN
                )
                # row reduce: TT-halving chain at 2x, then one short 1x reduce
                cur, w = rowrun, npts
                while w > 128:
                    w //= 2
                    nxt = redp.tile([128, w], FP16, tag=f"red{w}", name=f"red{w}")
                    nc.vector.tensor_tensor(
                        out=nxt[:], in0=cur[:, :w], in1=cur[:, w:], op=MIN
                    )
                    cur = nxt
                nc.vector.tensor_reduce(
                    out=rowmins[:, r : r + 1], in_=cur[:], axis=X, op=MIN
                )

        # ---------------- phase 2: column minima via PE transpose -----------
        # Two 128x128 transposes per PSUM tile, one [128,2,128] reduce each.
        with tc.tile_pool(name="pstp", bufs=4, space="PSUM") as pstp:
            for j in range(npts // 256):
                tp = pstp.tile([128, 2, 128], FP16, tag="tp")
                for h in range(2):
                    nc.tensor.transpose(
                        tp[:, h],
                        colacc[:, 256 * j + 128 * h : 256 * j + 128 * (h + 1)],
                        identity,
                    )
                nc.vector.tensor_reduce(
                    out=colmins[:, 2 * j : 2 * j + 2], in_=tp[:], axis=X, op=MIN
                )

        # ---------------- phase 3: sqrt + means ----------------------------
        with (
            tc.tile_pool(name="ps3", bufs=1, space="PSUM") as ps3,
            tc.tile_pool(name="fin", bufs=1) as finp,
        ):
            rmr = finp.tile([128, NRT], F32)
            nc.vector.tensor_scalar_max(rmr[:], rowmins[:], 0.0)
            cmr = finp.tile([128, NRT], F32)
            nc.vector.tensor_scalar_max(cmr[:], colmins[:], 0.0)
            rms = finp.tile([128, NRT], F32)
            nc.scalar.activation(rms[:], rmr[:], mybir.ActivationFunctionType.Sqrt)
            cms = finp.tile([128, NRT], F32)
            nc.scalar.activation(cms[:], cmr[:], mybir.ActivationFunctionType.Sqrt)
            s0 = finp.tile([128, 1], F32)
            nc.vector.tensor_reduce(out=s0[:], in_=rms[:], axis=X, op=ADD)
            s1 = finp.tile([128, 1], F32)
            nc.vector.tensor_reduce(out=s1[:], in_=cms[:], axis=X, op=ADD)
            s = finp.tile([128, 1], F32)
            nc.vector.tensor_tensor(out=s[:], in0=s0[:], in1=s1[:], op=ADD)
            pf = ps3.tile([1, 1], F32)
            nc.tensor.matmul(pf[:], s[:], ones[:], start=True, stop=True)
            res = finp.tile([1, 1], F32)
            nc.scalar.mul(res[:], pf[:], 1.0 / (2.0 * npts))
            nc.sync.dma_start(out=out_ext[:], in_=res[:])


@functools.lru_cache(maxsize=4)
def _build(npts, reps=1):
    import concourse.bacc as bacc
    import concourse.tile as tile
    from concourse import mybir

    nc = bacc.Bacc("TRN2", target_bir_lowering=False, debug=False)
    u_ext = nc.dram_tensor("u", [K, npts], mybir.dt.float16, kind="ExternalInput")
    v_ext = nc.dram_tensor("v", [K, npts], mybir.dt.float16, kind="ExternalInput")
    out_ext = nc.dram_tensor("out", [1, 1], mybir.dt.float32, kind="ExternalOutput")
    with tile.TileContext(nc) as tc:
        _emit(nc, tc, u_ext, v_ext, out_ext, npts, reps)
    nc.compile()
    return nc


def _run(pred_seq, tgt_output, npts=NPTS, trace=False, reps=1):
    from concourse.bass_utils import run_bass_kernel_spmd

    pred_seq = np.asarray(pred_seq, dtype=np.float32)
    tgt_output = np.asarray(tgt_output, dtype=np.float32)
    b = pred_seq.shape[0]
    nc = _build(npts, reps)
    in_maps = []
    for i in range(b):
        U, V = _augment(pred_seq[i], tgt_output[i])
        in_maps.append({"u": U, "v": V})
    res = run_bass_kernel_spmd(nc, in_maps, list(range(b)), trace=trace)
    out = np.array(
        [res.results[i]["out"][0, 0] for i in range(b)], dtype=np.float32
    )
    return out, res


def kernel(pred_seq, tgt_output):
    out, _ = _run(pred_seq, tgt_output)
    return out



# revision 15
# speedup vs baseline: 1.1097x; 1.1097x over previous
"""Chamfer loss kernel for Trainium2 (8 NeuronCores, data-parallel over batch).

Problem: pred_seq [8,8192,3] f32, tgt_output [8,8192,3] f32 ->
  chamfer [8] f32, where per batch b:
    d[n,m]   = || pred[b,n] - tgt[b,m] ||_2
    chamfer  = (mean_n min_m d + mean_m min_n d) / 2

Strategy (one batch element per core):
  - d2[n,m] = |p_n|^2 + |t_m|^2 - 2 p.t computed as ONE K=16 fp16 matmul per
    128x512 tile, using an exact fp16 hi/lo split of the coordinates (products
    of fp16 are exact in the PE's fp32 accumulator; measured d2 error ~7e-6).
  - min-before-sqrt: min_m sqrt(x) == sqrt(min_m x), so only the 2x8192 row/col
    minima ever need sqrt.
  - PSUM groups are consumed in pairs: ScalarE stages two 2048-wide f32 PSUM
    groups into one 4096-wide fp16 SBUF tile (pair A lands directly in the row
    running-min buffer), then VectorE runs 2x-rate fp16 tensor-tensor mins for
    the column accumulators and row running min, plus a TT-halving chain for
    the final free-axis row reduction (all at the DVE's 2-elem/cycle limit).
  - Column minima finish with PE transposes + free-axis reductions.
  - Means via ones-matmul partition sum; sqrt on the 2x8192 minima only.

Host side does only format prep: the fp16 hi/lo split + squared norms
(0.5 MFLOP total vs ~3.4 GFLOP on device).
"""

import functools
import sys

if "/opt/trn_rl_repo" not in sys.path:
    sys.path.insert(0, "/opt/trn_rl_repo")

import numpy as np

B = 8
NPTS = 8192
D = 3
K = 16  # augmented contraction dim: 4 slots per coord + 2 norm slots per side
BIG = 60000.0  # > max possible d2 (~200), fits fp16


# ---------------------------------------------------------------------------
# host-side augmentation: exact fp16 hi/lo split
# ---------------------------------------------------------------------------
def _split(x32):
    h = x32.astype(np.float16)
    l = (x32 - h.astype(np.float32)).astype(np.float16)
    return h, l


def _augment(pred, tgt):
    """pred/tgt: [N,3] f32 -> U,V [16,N] fp16 with d2 = (U^T V)[n,m]."""
    n = pred.shape[0]
    U = np.empty((K, n), np.float16)
    V = np.empty((K, n), np.float16)
    for d in range(D):
        hp, lp = _split(pred[:, d])
        ht, lt = _split(tgt[:, d])
        U[4 * d + 0] = hp
        U[4 * d + 1] = hp
        U[4 * d + 2] = lp
        U[4 * d + 3] = lp
        V[4 * d + 0] = -2.0 * ht
        V[4 * d + 1] = -2.0 * lt
        V[4 * d + 2] = -2.0 * ht
        V[4 * d + 3] = -2.0 * lt
    np_p = (pred * pred).sum(axis=1, dtype=np.float32)
    np_t = (tgt * tgt).sum(axis=1, dtype=np.float32)
    h, l = _split(np_p)
    U[12], U[13] = h, l
    V[12], V[13] = 1.0, 1.0
    h, l = _split(np_t)
    U[14], U[15] = 1.0, 1.0
    V[14], V[15] = h, l
    return U, V


# ---------------------------------------------------------------------------
# device program
# ---------------------------------------------------------------------------
def _emit(nc, tc, u_ext, v_ext, out_ext, npts, reps=1):
    from contextlib import nullcontext

    import concourse.tile as tile  # noqa: F401
    from concourse import mybir
    from concourse.masks import make_identity

    FP16 = mybir.dt.float16
    F32 = mybir.dt.float32
    MIN = mybir.AluOpType.min
    ADD = mybir.AluOpType.add
    X = mybir.AxisListType.X

    GROUP = 2048
    NG = npts // GROUP  # col groups
    NRT = npts // 128  # row tiles
    NC4 = GROUP // 512  # matmuls per group

    with (
        tc.tile_pool(name="consts", bufs=1) as consts,
        tc.tile_pool(name="uv", bufs=1) as uv,
        tc.tile_pool(name="acc", bufs=1) as accp,
        tc.tile_pool(name="mins", bufs=1) as minsp,
    ):
        identity = consts.tile([128, 128], FP16)
        make_identity(nc, identity)
        ones = consts.tile([128, 1], F32)
        nc.vector.memset(ones, 1.0)

        u = uv.tile([K, npts], FP16)
        nc.sync.dma_start(out=u, in_=u_ext[:])
        v = uv.tile([K, npts], FP16)
        # split the v load so the first matmul groups start sooner
        for g in range(4):
            sl = slice(g * (npts // 4), (g + 1) * (npts // 4))
            nc.sync.dma_start(out=v[:, sl], in_=v_ext[:, sl])

        colacc = accp.tile([128, npts], FP16, tag="colacc", name="colacc")

        rowmins = minsp.tile([128, NRT], F32)
        colmins = minsp.tile([128, NRT], F32)

        rep_cm = tc.For_i(0, reps, 1) if reps > 1 else nullcontext()
        with rep_cm:
            _emit_body(
                nc, tc, v, u, out_ext, colacc, rowmins, colmins, identity, ones, npts
            )


def _emit_body(nc, tc, v, u, out_ext, colacc, rowmins, colmins, identity, ones, npts):
    from concourse import mybir

    FP16 = mybir.dt.float16
    F32 = mybir.dt.float32
    MIN = mybir.AluOpType.min
    ADD = mybir.AluOpType.add
    X = mybir.AxisListType.X

    GROUP = 2048
    NG = npts // GROUP
    NRT = npts // 128
    NC4 = GROUP // 512

    if True:  # preserve indentation structure

        # ---------------- phase 1: d2 tiles + row/col min accumulation ------
        # ScalarE stages all NG 2048-wide PSUM groups of one row tile into a
        # single npts-wide fp16 SBUF tile. VectorE then needs just ONE
        # full-width 2x-rate tensor-tensor min into the column accumulator,
        # and the row min is a TT-halving chain (2x rate) off the same staged
        # tile + one short 1x reduce. 6 DVE ops per row tile, all at the
        # DVE's 2-elem/cycle crossbar limit.
        with (
            tc.tile_pool(name="psmm", bufs=2, space="PSUM") as psmm,
            tc.tile_pool(name="rows", bufs=3) as rowsp,
            tc.tile_pool(name="red", bufs=3) as redp,
        ):
            for r in range(NRT):
                lhsT = u[:, 128 * r : 128 * (r + 1)]
                rowrun = rowsp.tile([128, npts], FP16, tag="rowrun")
                for g in range(NG):
                    pg = psmm.tile([128, GROUP], F32, tag="mm")
                    for c in range(NC4):
                        nc.tensor.matmul(
                            pg[:, 512 * c : 512 * (c + 1)],
                            lhsT,
                            v[:, GROUP * g + 512 * c : GROUP * g + 512 * (c + 1)],
                            start=True,
                            stop=True,
                        )
                    nc.scalar.copy(rowrun[:, GROUP * g : GROUP * (g + 1)], pg[:])
                    if r == 0:
                        # first tile initializes colacc with 4x-rate copies
                        # (per group, so they start as soon as each lands)
                        nc.vector.tensor_copy(
                            colacc[:, GROUP * g : GROUP * (g + 1)],
                            rowrun[:, GROUP * g : GROUP * (g + 1)],
                        )
                # column accumulator (elementwise min across row tiles); the
                # last tile updates in 1024-wide slices so phase 2 can begin
                # per column range.
                if r == 0:
                    pass
                elif r == NRT - 1:
                    for s in range(8):
                        sl = slice(1024 * s, 1024 * (s + 1))
                        nc.vector.tensor_tensor(
                            out=colacc[:, sl], in0=rowrun[:, sl], in1=colacc[:, sl],
                            op=MIN,
                        )
                else:
                    nc.vector.tensor_tensor(
                        out=colacc[:], in0=rowrun[:], in1=colacc[:], op=MIN
                    )
                # row reduce: TT-halving chain at 2x down to 256, then one
                # 1x reduce (cheaper than chaining all the way to 128).
                cur, w = rowrun, npts
                while w > 256:
                    w //= 2
                    nxt = redp.tile([128, w], FP16, tag=f"red{w}", name=f"red{w}")
                    nc.vector.tensor_tensor(
                        out=nxt[:], in0=cur[:, :w], in1=cur[:, w:], op=MIN
                    )
                    cur = nxt
                nc.vector.tensor_reduce(
                    out=rowmins[:, r : r + 1], in_=cur[:], axis=X, op=MIN
                )

        # ---------------- phase 2: column minima via PE transpose -----------
        # Eight 128x128 transposes per PSUM tile, one [128,8,128] reduce each
        # (wider reduces amortize the 1x-op init cost).
        with tc.tile_pool(name="pstp", bufs=4, space="PSUM") as pstp:
            for j in range(npts // 1024):
                tp = pstp.tile([128, 8, 128], FP16, tag="tp")
                for h in range(8):
                    nc.tensor.transpose(
                        tp[:, h],
                        colacc[:, 1024 * j + 128 * h : 1024 * j + 128 * (h + 1)],
                        identity,
                    )
                nc.vector.tensor_reduce(
                    out=colmins[:, 8 * j : 8 * j + 8], in_=tp[:], axis=X, op=MIN
                )

        # ---------------- phase 3: sqrt + means ----------------------------
        with (
            tc.tile_pool(name="ps3", bufs=1, space="PSUM") as ps3,
            tc.tile_pool(name="fin", bufs=1) as finp,
        ):
            rmr = finp.tile([128, NRT], F32)
            nc.vector.tensor_scalar_max(rmr[:], rowmins[:], 0.0)
            cmr = finp.tile([128, NRT], F32)
            nc.vector.tensor_scalar_max(cmr[:], colmins[:], 0.0)
            rms = finp.tile([128, NRT], F32)
            nc.scalar.activation(rms[:], rmr[:], mybir.ActivationFunctionType.Sqrt)
            cms = finp.tile([128, NRT], F32)
            nc.scalar.activation(cms[:], cmr[:], mybir.ActivationFunctionType.Sqrt)
            s0 = finp.tile([128, 1], F32)
            nc.vector.tensor_reduce(out=s0[:], in_=rms[:], axis=X, op=ADD)
            s1 = finp.tile([128, 1], F32)
            nc.vector.tensor_reduce(out=s1[:], in_=cms[:], axis=X, op=ADD)
            s = finp.tile([128, 1], F32)
            nc.vector.tensor_tensor(out=s[:], in0=s0[:], in1=s1[:], op=ADD)
            pf = ps3.tile([1, 1], F32)
            nc.tensor.matmul(pf[:], s[:], ones[:], start=True, stop=True)
            res = finp.tile([1, 1], F32)
            nc.scalar.mul(res[:], pf[:], 1.0 / (2.0 * npts))
            nc.sync.dma_start(out=out_ext[:], in_=res[:])


@functools.lru_cache(maxsize=4)
def _build(npts, reps=1):
    import concourse.bacc as bacc
    import concourse.tile as tile
    from concourse import mybir

    nc = bacc.Bacc("TRN2", target_bir_lowering=False, debug=False)
    u_ext = nc.dram_tensor("u", [K, npts], mybir.dt.float16, kind="ExternalInput")
    v_ext = nc.dram_tensor("v", [K, npts], mybir.dt.float16, kind="ExternalInput")
    out_ext = nc.dram_tensor("out", [1, 1], mybir.dt.float32, kind="ExternalOutput")
    with tile.TileContext(nc) as tc:
        _emit(nc, tc, u_ext, v_ext, out_ext, npts, reps)
    nc.compile()
    return nc


def _run(pred_seq, tgt_output, npts=NPTS, trace=False, reps=1):
    from concourse.bass_utils import run_bass_kernel_spmd

    pred_seq = np.asarray(pred_seq, dtype=np.float32)
    tgt_output = np.asarray(tgt_output, dtype=np.float32)
    b = pred_seq.shape[0]
    nc = _build(npts, reps)
    in_maps = []
    for i in range(b):
        U, V = _augment(pred_seq[i], tgt_output[i])
        in_maps.append({"u": U, "v": V})
    res = run_bass_kernel_spmd(nc, in_maps, list(range(b)), trace=trace)
    out = np.array(
        [res.results[i]["out"][0, 0] for i in range(b)], dtype=np.float32
    )
    return out, res


def kernel(pred_seq, tgt_output):
    out, _ = _run(pred_seq, tgt_output)
    return out

